# revision 26
# baseline (speedup 1.0000x reference)
"""GQA attention decode kernel for Trainium2 (Bass/Tile), SPMD over 8 NeuronCores.

Sharding: kv-head axis (K=2) x batch groups (4) -> 8 cores.
Core c: kv head k=c%2, batches [2*(c//2), 2*(c//2)+2).
Each core computes q/k/v projections + RoPE for its head group, attends over
its shard of the KV cache (only rows [0, cur_ind+T) ever contribute), and
produces a partial output projection. Host sums the two kv-head partials.

All heavy tensors stream as bf16 (host casts during sharding); PSUM
accumulation stays fp32. The K cache is pre-transposed on host to [H, S]
per batch so the hot loop needs no PE transposes; the V cache is packed
block-major [128, NB, H+1] with a ones column folded in (softmax
denominator accumulates alongside the numerator in one matmul chain).

Every DMA rides the single SP HWDGE ring in need order (ring order ==
completion order), so compute overlaps the stream: x/rope first, then wq
in chunks (projections start ~4us in), batch-0 cache, wk/wv, batch-1
cache, wo per-head (output projection consumes chunks as they land).
Exp is batched 4 s-blocks per activation to amortize the ~175ns fixed
Act-engine cost, with the PE issue order software-pipelined so the next
quad's logits run while the previous quad's exp is in flight. The RoPE
half-swap is a PE permutation matmul (no SBUF->SBUF DMA on the hot ring).

Shapes (hardcoded from the problem spec):
  x [8,16,1536], k_cache/v_cache [8,8192,2,128],
  wq [1536,12,128], wk/wv [1536,2,128], wo [12,128,1536], out [8,16,1536]
"""

import sys

if "/opt/trn_rl_repo" not in sys.path:
    sys.path.insert(0, "/opt/trn_rl_repo")

import numpy as np
import ml_dtypes

BF16 = np.dtype(ml_dtypes.bfloat16)
FP8 = np.dtype(ml_dtypes.float8_e4m3)

B, T, S, D, N, K, H = 8, 16, 8192, 1536, 12, 2, 128
G = N // K            # 6 q heads per kv head
BG = 4                # batch groups
BL = B // BG          # 2 local batches per core
DC = D // 128         # 12 contraction chunks
QD = 4                # s-blocks per exp batch (4*96 fp32 = 1.5KB of a PSUM bank)
ROPE_THETA = 1000000.0
NEG = -1.0e30

_built = {}


# ---------------------------------------------------------------- host math
def _host_rope(positions):
    # positions [b, t] int32 -> sin, cos [b, t, 64] float32 (mirrors reference)
    frac = np.arange(0, H, 2, dtype=np.float32) / np.float32(H)
    timescale = np.power(np.float32(ROPE_THETA), frac, dtype=np.float32)
    ang = positions[..., None].astype(np.float32) / timescale
    return np.sin(ang, dtype=np.float32), np.cos(ang, dtype=np.float32)


def _host_mask(segment_ids, start_ind, cur):
    seg = np.asarray(segment_ids, np.int32)
    sti = np.asarray(start_ind, np.int32)
    nonpad = seg != 0
    left_pads = np.argmax(nonpad, axis=-1).astype(np.int32)
    start = np.where(sti < 0, left_pads, sti).astype(np.int32)
    positions = np.maximum(np.cumsum(nonpad.astype(np.int32), axis=-1) - 1, 0) + cur

    q_pos = cur + np.arange(T, dtype=np.int32)[None, :] - start[:, None]
    ts_ = np.arange(S, dtype=np.int32)
    kv_seg = (ts_[None, :] >= start[:, None]) & (ts_[None, :] < cur + T)
    k_pos = ts_[None, :] - start[:, None]
    causal = k_pos[:, None, :] <= q_pos[:, :, None]
    segm = kv_seg[:, None, :].astype(np.int32) == seg[:, :, None]
    mask = causal & segm  # [b, t, S] True = attend
    return mask, positions


def _numpy_reference(x, k_cache, v_cache, wq, bq, wk, bk, wv, bv, wo,
                     segment_ids, start_ind, cur):
    # Full-precision numpy fallback (used only for inputs outside the
    # spec envelope: non-zero biases, odd cur_ind alignment, pad tokens).
    mask, positions = _host_mask(segment_ids, start_ind, cur)
    sin, cos = _host_rope(positions)

    def rope(t):  # t [b,tk,n,h]
        h2 = H // 2
        x1, x2 = t[..., :h2], t[..., h2:]
        s = sin[:, :, None, :]
        c = cos[:, :, None, :]
        return np.concatenate([x1 * c - x2 * s, x2 * c + x1 * s], axis=-1)

    q = np.einsum("btd,dnh->btnh", x, wq) + bq
    kp = np.einsum("btd,dkh->btkh", x, wk) + bk
    v = np.einsum("btd,dkh->btkh", x, wv) + bv
    q = rope(q)
    kp = rope(kp)
    kc = np.array(k_cache)
    vc = np.array(v_cache)
    kc[:, cur:cur + T] = kp
    vc[:, cur:cur + T] = v
    scale = np.float32(H) ** -0.5
    qg = q.reshape(B, T, K, G, H)
    logits = np.einsum("btkgh,bskh->btskg", qg, kc) * scale
    logits = np.where(mask[:, :, :, None, None], logits, np.float32(-3.3895314e38))
    logits = logits - logits.max(axis=2, keepdims=True)
    w = np.exp(logits.astype(np.float32))
    w = w / w.sum(axis=2, keepdims=True)
    qkv = np.einsum("btskg,bskh->btkgh", w, vc).reshape(B, T, N, H)
    return np.einsum("btnh,nhd->btd", qkv, wo).astype(np.float32)


# ---------------------------------------------------------------- device build
def _build(sold):
    import concourse.bass as bass
    import concourse.bacc as bacc
    import concourse.tile as tile
    from concourse import mybir
    from concourse.masks import make_identity

    f32 = mybir.dt.float32
    bf = mybir.dt.bfloat16
    f8 = mybir.dt.float8e4
    NB = sold // 128      # 128-row s blocks per batch
    NQ = (NB + QD - 1) // QD
    BT = BL * T  # 32
    # one consolidated bf16 "smalls" tensor: swap matrix | ropeq | ropek | xT | nmask
    SW0, SW1 = 0, 128
    RQ0, RQ1 = 128, 128 + 2 * BT              # 192
    RK0, RK1 = RQ1, RQ1 + 2 * BT              # 256
    XT0, XT1 = RK1, RK1 + DC * BT             # 640
    NM0, NM1 = XT1, XT1 + BL * G * T          # 832
    SM = NM1

    nc = bacc.Bacc(None)
    smalld = nc.declare_dram_parameter("smalls", [128, SM], bf, isOutput=False)
    wqk = nc.declare_dram_parameter("wqk", [128, DC, G * H], bf, isOutput=False)
    wkv = nc.declare_dram_parameter("wkv", [128, DC, 2 * H], f8, isOutput=False)
    wok = nc.declare_dram_parameter("wok", [128, G, D], bf, isOutput=False)
    kcp = nc.declare_dram_parameter("kcp", [BL, 128, sold], f8, isOutput=False)
    vcp = nc.declare_dram_parameter("vcp", [BL, 128, NB, H + 1], bf, isOutput=False)
    outp = nc.declare_dram_parameter("out", [BT, D], f32, isOutput=True)

    with tile.TileContext(nc) as tc:
        with (
            tc.tile_pool(name="cpool", bufs=1) as cpool,
            tc.tile_pool(name="wtpool", bufs=4) as wtp,
            tc.tile_pool(name="spool", bufs=2) as sp,
            tc.tile_pool(name="pl", bufs=3, space="PSUM") as pl,
            tc.tile_pool(name="pacc", bufs=1, space="PSUM") as pacc,
            tc.tile_pool(name="pp", bufs=3, space="PSUM") as pp,
        ):
            ident = cpool.tile([128, 128], bf)
            make_identity(nc, ident)

            # ---- ALL loads on the SP HWDGE ring, strictly in need order,
            # consolidated to few triggers (each costs ~650ns on the SP seq).
            smalls = cpool.tile([128, SM], bf)
            nc.sync.dma_start(out=smalls, in_=smalld[:])
            swp = smalls[:, SW0:SW1]
            rq_s = smalls[:, RQ0:RQ1].rearrange("p (a t) -> p a t", a=2)
            rk_t = smalls[:, RK0:RK1].rearrange("p (a t) -> p a t", a=2)
            xT = smalls[:, XT0:XT1].rearrange("p (c t) -> p c t", c=DC)
            nm_t = smalls[0:T, NM0:NM1].rearrange("p (l gt) -> p l gt", l=BL)
            # wq chunked so projections start as soon as the first chunk lands
            wq_t = cpool.tile([128, DC, G * H], bf)
            for c0, c1 in ((0, 2), (2, 4), (4, 8), (8, 12)):
                nc.sync.dma_start(out=wq_t[:, c0:c1, :], in_=wqk[:, c0:c1, :])
            kT_all = cpool.tile([128, BL, sold], f8)
            vB_all = cpool.tile([128, BL, NB, H + 1], bf)
            wkv_t = cpool.tile([128, DC, 2 * H], f8)
            nc.sync.dma_start(out=wkv_t, in_=wkv[:])
            wk_t = wkv_t[:, :, 0:H]
            wv_t = wkv_t[:, :, H:2 * H]
            nc.sync.dma_start(out=kT_all[:, 0, :], in_=kcp[0])
            nc.sync.dma_start(out=vB_all[:, 0, :, :], in_=vcp[0])
            nc.sync.dma_start(out=kT_all[:, 1, :], in_=kcp[1])
            nc.sync.dma_start(out=vB_all[:, 1, :, :], in_=vcp[1])
            # wo halves: output projection consumes as chunks land
            wo_t = cpool.tile([128, G, D], bf)
            for cc in range(2):
                nc.sync.dma_start(out=wo_t[:, 3 * cc:3 * cc + 3, :],
                                  in_=wok[:, 3 * cc:3 * cc + 3, :])

            # ---- preload the Act EXP table off the critical path
            scr = cpool.tile([1, 4], f32)
            nc.vector.memset(scr, 0.0)
            scrE = cpool.tile([1, 4], bf)
            nc.scalar.activation(scrE, scr, mybir.ActivationFunctionType.Exp)

            # rope-q coeffs broadcast across heads (DVE is idle this early)
            rq_t = cpool.tile([128, 2, G, BT], bf)
            for g in range(G):
                nc.vector.tensor_copy(rq_t[:, :, g, :], rq_s)

            # ---- projections (accumulate over DC chunks)
            qp0 = pp.tile([BT, 512], f32, tag="pp")
            qp1 = pp.tile([BT, 256], f32, tag="pp")
            for c in range(DC):
                st, spf = (c == 0), (c == DC - 1)
                nc.tensor.matmul(qp0, xT[:, c, :], wq_t[:, c, 0:512], start=st, stop=spf)
                nc.tensor.matmul(qp1, xT[:, c, :], wq_t[:, c, 512:768], start=st, stop=spf)
            q_sb = cpool.tile([BT, G * H], bf)
            # ---- q -> qT, half-swap via PE permutation, RoPE (scale in coeffs)
            # per-head copies + transposes so the chain pipelines across
            # DVE and PE instead of serializing on full-tensor copies
            qTr = cpool.tile([128, G, BT], bf)
            for g in range(G):
                src = qp0[:, g * H:(g + 1) * H] if g < 4 else                     qp1[:, (g - 4) * H:(g - 3) * H]
                nc.vector.tensor_copy(q_sb[:, g * H:(g + 1) * H], src)
                tp = pl.tile([128, BT], bf, tag="pl")
                nc.tensor.transpose(tp, q_sb[:, g * H:(g + 1) * H], ident[:BT, :BT])
                nc.vector.tensor_copy(qTr[:, g, :], tp)
            qR = cpool.tile([128, G, BT], bf)
            qtmp = cpool.tile([128, G, BT], bf)
            nc.vector.tensor_tensor(qtmp, qTr, rq_t[:, 0], mybir.AluOpType.mult)
            qSwP = pl.tile([128, G * BT], f32, tag="pl", name="qswp")
            nc.tensor.matmul(qSwP, swp, qTr.rearrange("h g t -> h (g t)"),
                             start=True, stop=True)
            nc.vector.tensor_tensor(
                qR, qSwP.rearrange("h (g t) -> h g t", g=G), rq_t[:, 1],
                mybir.AluOpType.mult)
            nc.vector.tensor_tensor(qR, qR, qtmp, mybir.AluOpType.add)

            # ---- attention hot loop: per 128-row s block one logits matmul
            # (pre-transposed K stationary) and one qkv accumulate; exp runs
            # once per quad of blocks. PE issue is software-pipelined: quad
            # q's logits go out before quad q-1's qkv so PE never waits on Act.
            # The k/v projections + new-token block run between the two batch
            # loops (their wkv weights land after the caches in the stream);
            # the new-token matmuls close each accumulation group at the end.
            qkvT = cpool.tile([128, G, BT], bf)
            qkvPs = [pacc.tile([G * T, H + 1], f32, tag="pacc", name=f"qkvP{lb}")
                     for lb in range(BL)]

            def hot_loop(lb):
                qkvP = qkvPs[lb]
                qrs = qR[:, :, lb * T:(lb + 1) * T]
                pend = []   # [(wTq, q0, nqd)] awaiting qkv, depth 2
                first = [True]

                def flush_one():
                    pw, p0, pn = pend.pop(0)
                    for j in range(pn):
                        nc.tensor.matmul(qkvP, pw[:, j, :],
                                         vB_all[:, lb, p0 + j, :],
                                         start=first[0] and j == 0, stop=False,
                                         skip_group_check=True)
                    first[0] = False

                for q in range(NQ):
                    q0 = q * QD
                    nqd = min(QD, NB - q0)
                    lps = pl.tile([128, QD, G * T], f32, tag="pl")
                    for j in range(nqd):
                        blk = q0 + j
                        nc.tensor.matmul(lps[:, j, :],
                                         kT_all[:, lb, blk * 128:(blk + 1) * 128],
                                         qrs, start=True, stop=True,
                                         skip_group_check=True)
                    wTq = wtp.tile([128, QD, G * T], bf, tag="wt")
                    if nqd == QD:
                        nc.scalar.activation(wTq, lps,
                                             mybir.ActivationFunctionType.Exp)
                    else:
                        nc.scalar.activation(wTq[:, 0:nqd, :], lps[:, 0:nqd, :],
                                             mybir.ActivationFunctionType.Exp)
                    pend.append((wTq, q0, nqd))
                    if len(pend) > 2:
                        flush_one()
                while pend:
                    flush_one()

            # ---- k_new projection -> kT + RoPE (no scale)
            kp = pp.tile([BT, H], f32, tag="pp")
            for c in range(DC):
                nc.tensor.matmul(kp, xT[:, c, :], wk_t[:, c, :],
                                 start=(c == 0), stop=(c == DC - 1))
            k_sb = cpool.tile([BT, H], bf)
            nc.vector.tensor_copy(k_sb, kp)
            kTr = cpool.tile([128, BT], bf)
            tpk = pl.tile([128, BT], bf, tag="pl")
            nc.tensor.transpose(tpk, k_sb, ident[:BT, :BT])
            nc.vector.tensor_copy(kTr, tpk)
            kSwP = pl.tile([128, BT], f32, tag="pl", name="kswp")
            nc.tensor.matmul(kSwP, swp, kTr, start=True, stop=True)
            kR = cpool.tile([128, BT], bf)
            ktmp = cpool.tile([128, BT], bf)
            nc.vector.tensor_tensor(ktmp, kTr, rk_t[:, 0], mybir.AluOpType.mult)
            nc.vector.tensor_tensor(kR, kSwP, rk_t[:, 1], mybir.AluOpType.mult)
            nc.vector.tensor_tensor(kR, kR, ktmp, mybir.AluOpType.add)

            # ---- v_new projection
            vN = cpool.tile([T, BL, H + 1], bf)
            for lb in range(BL):
                vp = pp.tile([T, H], f32, tag="pp", name=f"vp{lb}")
                for c in range(DC):
                    nc.tensor.matmul(vp, xT[:, c, lb * T:(lb + 1) * T], wv_t[:, c, :],
                                     start=(c == 0), stop=(c == DC - 1))
                nc.vector.tensor_copy(vN[:, lb, 0:H], vp)
            nc.vector.memset(vN[:, :, H:H + 1], 1.0)

            # ---- new-token logits (kv rows [cur, cur+T) live on-chip)
            wTns = []
            for lb in range(BL):
                lpn = pl.tile([T, G * T], f32, tag="pl", name=f"lpn{lb}")
                nc.tensor.matmul(lpn, kR[:, lb * T:(lb + 1) * T],
                                 qR[:, :, lb * T:(lb + 1) * T],
                                 start=True, stop=True)
                nc.vector.tensor_tensor(lpn, lpn, nm_t[:, lb, :], mybir.AluOpType.add)
                wTn = sp.tile([T, G * T], bf, tag="wtn", name=f"wTn{lb}")
                nc.scalar.activation(wTn, lpn, mybir.ActivationFunctionType.Exp)
                wTns.append(wTn)

            hot_loop(0)

            hot_loop(1)

            # new tokens close both accumulation groups
            for lb in range(BL):
                nc.tensor.matmul(qkvPs[lb], wTns[lb], vN[:, lb, :],
                                 start=False, stop=True, skip_group_check=True)

            # ---- epilogues after both batches
            for lb in range(BL):
                qkvP = qkvPs[lb]
                rec = sp.tile([G * T, 1], f32, tag="rec")
                nc.vector.reciprocal(rec, qkvP[:, H:H + 1])
                qkvN = sp.tile([G * T, H], bf, tag="qkvN")
                nc.vector.tensor_scalar_mul(qkvN, qkvP[:, 0:H], rec)
                tp3 = pl.tile([128, G * T], bf, tag="pl")
                nc.tensor.transpose(tp3, qkvN, ident[:G * T, :G * T])
                nc.vector.tensor_copy(
                    qkvT[:, :, lb * T:(lb + 1) * T],
                    tp3.rearrange("h (g t) -> h g t", g=G))

            # ---- output projection: out[bt, d] = sum_g qkvT[h,g,bt]^T wo[h,g,d]
            o_sb = cpool.tile([BT, D], f32)
            for db in range(3):
                oP = pp.tile([BT, 512], f32, tag="pp", name=f"oP{db}")
                for g in range(G):
                    nc.tensor.matmul(oP, qkvT[:, g, :],
                                     wo_t[:, g, db * 512:(db + 1) * 512],
                                     start=(g == 0), stop=(g == G - 1))
                if db == 1:
                    nc.scalar.activation(o_sb[:, db * 512:(db + 1) * 512], oP,
                                         mybir.ActivationFunctionType.Copy)
                else:
                    nc.vector.tensor_copy(o_sb[:, db * 512:(db + 1) * 512], oP)
                nc.sync.dma_start(out=outp[:, db * 512:(db + 1) * 512],
                                  in_=o_sb[:, db * 512:(db + 1) * 512])

    nc.compile()  # bacc passes: splits multi-wait instructions (TRN2 allows 1)
    return nc


# ---------------------------------------------------------------- entry point
def kernel(x, k_cache, v_cache, wq, bq, wk, bk, wv, bv, wo,
           segment_ids, start_ind, cur_ind):
    x = np.asarray(x, np.float32)
    k_cache = np.asarray(k_cache, np.float32)
    v_cache = np.asarray(v_cache, np.float32)
    wq = np.asarray(wq, np.float32)
    wk = np.asarray(wk, np.float32)
    wv = np.asarray(wv, np.float32)
    wo = np.asarray(wo, np.float32)
    cur = int(np.asarray(cur_ind))

    mask, positions = _host_mask(segment_ids, start_ind, cur)

    spec_ok = (
        cur % 128 == 0 and 0 < cur and cur + T <= S
        and not np.any(np.asarray(bq)) and not np.any(np.asarray(bk))
        and not np.any(np.asarray(bv))
        and not np.any(mask[:, :, cur + T:])          # nothing attended past new rows
        and bool(np.all(np.any(mask, axis=2)))        # no fully-masked query row
        and bool(np.all(mask[:, :, :cur]))            # all old-cache rows attended
    )
    if not spec_ok:
        return _numpy_reference(x, k_cache, v_cache, wq, bq, wk, bk, wv, bv, wo,
                                segment_ids, start_ind, cur)

    sold = cur
    key = sold
    if key not in _built:
        _built[key] = _build(sold)
    nc = _built[key]

    inputs = dict(x=x, k_cache=k_cache, v_cache=v_cache, wq=wq, wk=wk, wv=wv,
                  wo=wo, segment_ids=segment_ids, start_ind=start_ind,
                  cur_ind=cur)
    in_maps = _make_in_maps(inputs, sold, mask=mask, positions=positions)

    global _last_in_maps
    _last_in_maps = in_maps

    import os
    from concourse.bass_utils import run_bass_kernel_spmd
    trace = os.environ.get("KERNEL_TRACE", "0") == "1"
    res = run_bass_kernel_spmd(nc, in_maps, core_ids=list(range(8)), trace=trace)
    if trace and res.exec_time_ns is not None:
        print(f"HW exec time: {res.exec_time_ns} ns")

    out = np.zeros((B, T, D), np.float32)
    for c in range(8):
        bg = c // 2
        out[bg * BL:(bg + 1) * BL] += res.results[c]["out"].reshape(BL, T, D)
    return out


def _bf(a):
    return np.ascontiguousarray(a, dtype=BF16)


def _make_in_maps(inputs, sold, mask=None, positions=None):
    x = np.asarray(inputs["x"], np.float32)
    k_cache = np.asarray(inputs["k_cache"], np.float32)
    v_cache = np.asarray(inputs["v_cache"], np.float32)
    wq = np.asarray(inputs["wq"], np.float32)
    wk = np.asarray(inputs["wk"], np.float32)
    wv = np.asarray(inputs["wv"], np.float32)
    wo = np.asarray(inputs["wo"], np.float32)
    cur = int(np.asarray(inputs["cur_ind"]))
    NB = sold // 128
    BT = BL * T
    if mask is None:
        mask, positions = _host_mask(inputs["segment_ids"], inputs["start_ind"], cur)

    sin, cos = _host_rope(positions)  # [b, t, 64]
    scale = np.float32(H ** -0.5)

    # rope coeff layouts: rows h<64 -> (cos, -sin); h>=64 -> (cos, +sin)
    def rope_pack(bsl, ncols_g, with_scale):
        # returns [128, 2, ncols_g, BL*T]
        cs = cos[bsl]  # [BL, T, 64]
        sn = sin[bsl]
        ccol = np.transpose(cs, (2, 0, 1)).reshape(64, BL * T)  # [64, (b,t)]
        scol = np.transpose(sn, (2, 0, 1)).reshape(64, BL * T)
        top_c, bot_c = ccol, ccol
        top_s, bot_s = -scol, scol
        c128 = np.concatenate([top_c, bot_c], axis=0)   # [128, BT]
        s128 = np.concatenate([top_s, bot_s], axis=0)
        if with_scale:
            c128 = c128 * scale
            s128 = s128 * scale
        pack = np.stack([c128, s128], axis=1)           # [128, 2, BT]
        pack = np.repeat(pack[:, :, None, :], ncols_g, axis=2)
        return _bf(pack)

    # half-swap permutation: out[p, :] = in[(p + 64) % 128, :]
    swpm = np.zeros((128, 128), np.float32)
    swpm[(np.arange(128) + 64) % 128, np.arange(128)] = 1.0

    ones_col = np.ones((BL, sold, 1), np.float32)
    in_maps = []
    for c in range(8):
        k = c % 2
        bg = c // 2
        bsl = slice(bg * BL, (bg + 1) * BL)
        # x pre-transposed to contraction-major: [128, DC, BT]
        xT = x[bsl].reshape(BT, DC, 128).transpose(2, 1, 0)
        # weights in SBUF layout (partition = contraction chunk row)
        wq4 = wq.reshape(DC, 128, N, H)[:, :, k * G:(k + 1) * G, :] \
                .reshape(DC, 128, G * H).transpose(1, 0, 2)
        wk4 = wk.reshape(DC, 128, K, H)[:, :, k, :].transpose(1, 0, 2)
        wv4 = wv.reshape(DC, 128, K, H)[:, :, k, :].transpose(1, 0, 2)
        wkv4 = np.concatenate([wk4, wv4], axis=-1)       # [128, DC, 2H]
        wo4 = wo[k * G:(k + 1) * G].transpose(1, 0, 2)   # [128h, G, D]
        # K cache pre-transposed to [BL, H, sold]
        kcs = k_cache[bsl, :sold, k, :].transpose(0, 2, 1)
        # V cache block-major: [BL, 128, NB, H+1], s = blk*128 + p
        vcs = np.concatenate([v_cache[bsl, :sold, k, :], ones_col], axis=-1)
        vcs = vcs.reshape(BL, NB, 128, H + 1).transpose(0, 2, 1, 3)
        # additive mask for the new-token block: [T(s_new), BL, G*T]
        nm = np.where(mask[bsl][:, :, cur:cur + T], np.float32(0), np.float32(NEG))
        nm = np.transpose(nm, (2, 0, 1))                 # [s_new, BL, t]
        nm = np.repeat(nm[:, :, None, :], G, axis=2).reshape(T, BL, G * T)
        # consolidated smalls pack [128, SM] bf16
        BT_ = BL * T
        smalls = np.zeros((128, 832), np.float32)
        smalls[:, 0:128] = swpm
        smalls[:, 128:192] = np.asarray(
            rope_pack(bsl, 1, True), np.float32).reshape(128, 2 * BT_)
        smalls[:, 192:256] = np.asarray(
            rope_pack(bsl, 1, False), np.float32).reshape(128, 2 * BT_)
        smalls[:, 256:640] = xT.reshape(128, DC * BT_)
        smalls[0:T, 640:832] = nm.reshape(T, BL * G * T)
        in_maps.append({
            "smalls": _bf(smalls),
            "wqk": _bf(wq4),
            "wkv": np.ascontiguousarray(wkv4, dtype=FP8),
            "wok": _bf(wo4),
            "kcp": np.ascontiguousarray(kcs, dtype=FP8),
            "vcp": _bf(vcs),
        })

    return in_maps


# revision 27
# speedup vs baseline: 1.1043x; 1.1043x over previous
"""GQA attention decode kernel for Trainium2 (Bass/Tile), SPMD over 8 NeuronCores.

Sharding: kv-head axis (K=2) x batch groups (4) -> 8 cores.
Core c: kv head k=c%2, batches [2*(c//2), 2*(c//2)+2).
Each core computes q/k/v projections + RoPE for its head group, attends over
its shard of the KV cache (only rows [0, cur_ind+T) ever contribute), and
produces a partial output projection. Host sums the two kv-head partials.

All heavy tensors stream as bf16 (host casts during sharding); PSUM
accumulation stays fp32. The K cache is pre-transposed on host to [H, S]
per batch so the hot loop needs no PE transposes; the V cache is packed
block-major [128, NB, H+1] with a ones column folded in (softmax
denominator accumulates alongside the numerator in one matmul chain).

Every DMA rides the single SP HWDGE ring in need order (ring order ==
completion order), so compute overlaps the stream: x/rope first, then wq
in chunks (projections start ~4us in), batch-0 cache, wk/wv, batch-1
cache, wo per-head (output projection consumes chunks as they land).
Exp is batched 4 s-blocks per activation to amortize the ~175ns fixed
Act-engine cost, with the PE issue order software-pipelined so the next
quad's logits run while the previous quad's exp is in flight. The RoPE
half-swap is a PE permutation matmul (no SBUF->SBUF DMA on the hot ring).

Shapes (hardcoded from the problem spec):
  x [8,16,1536], k_cache/v_cache [8,8192,2,128],
  wq [1536,12,128], wk/wv [1536,2,128], wo [12,128,1536], out [8,16,1536]
"""

import sys

if "/opt/trn_rl_repo" not in sys.path:
    sys.path.insert(0, "/opt/trn_rl_repo")

import numpy as np
import ml_dtypes

BF16 = np.dtype(ml_dtypes.bfloat16)
FP8 = np.dtype(ml_dtypes.float8_e4m3)

B, T, S, D, N, K, H = 8, 16, 8192, 1536, 12, 2, 128
G = N // K            # 6 q heads per kv head
BG = 4                # batch groups
BL = B // BG          # 2 local batches per core
DC = D // 128         # 12 contraction chunks
QD = 4                # s-blocks per exp batch (4*96 fp32 = 1.5KB of a PSUM bank)
ROPE_THETA = 1000000.0
NEG = -1.0e30

_built = {}


# ---------------------------------------------------------------- host math
def _host_rope(positions):
    # positions [b, t] int32 -> sin, cos [b, t, 64] float32 (mirrors reference)
    frac = np.arange(0, H, 2, dtype=np.float32) / np.float32(H)
    timescale = np.power(np.float32(ROPE_THETA), frac, dtype=np.float32)
    ang = positions[..., None].astype(np.float32) / timescale
    return np.sin(ang, dtype=np.float32), np.cos(ang, dtype=np.float32)


def _host_mask(segment_ids, start_ind, cur):
    seg = np.asarray(segment_ids, np.int32)
    sti = np.asarray(start_ind, np.int32)
    nonpad = seg != 0
    left_pads = np.argmax(nonpad, axis=-1).astype(np.int32)
    start = np.where(sti < 0, left_pads, sti).astype(np.int32)
    positions = np.maximum(np.cumsum(nonpad.astype(np.int32), axis=-1) - 1, 0) + cur

    q_pos = cur + np.arange(T, dtype=np.int32)[None, :] - start[:, None]
    ts_ = np.arange(S, dtype=np.int32)
    kv_seg = (ts_[None, :] >= start[:, None]) & (ts_[None, :] < cur + T)
    k_pos = ts_[None, :] - start[:, None]
    causal = k_pos[:, None, :] <= q_pos[:, :, None]
    segm = kv_seg[:, None, :].astype(np.int32) == seg[:, :, None]
    mask = causal & segm  # [b, t, S] True = attend
    return mask, positions


def _numpy_reference(x, k_cache, v_cache, wq, bq, wk, bk, wv, bv, wo,
                     segment_ids, start_ind, cur):
    # Full-precision numpy fallback (used only for inputs outside the
    # spec envelope: non-zero biases, odd cur_ind alignment, pad tokens).
    mask, positions = _host_mask(segment_ids, start_ind, cur)
    sin, cos = _host_rope(positions)

    def rope(t):  # t [b,tk,n,h]
        h2 = H // 2
        x1, x2 = t[..., :h2], t[..., h2:]
        s = sin[:, :, None, :]
        c = cos[:, :, None, :]
        return np.concatenate([x1 * c - x2 * s, x2 * c + x1 * s], axis=-1)

    q = np.einsum("btd,dnh->btnh", x, wq) + bq
    kp = np.einsum("btd,dkh->btkh", x, wk) + bk
    v = np.einsum("btd,dkh->btkh", x, wv) + bv
    q = rope(q)
    kp = rope(kp)
    kc = np.array(k_cache)
    vc = np.array(v_cache)
    kc[:, cur:cur + T] = kp
    vc[:, cur:cur + T] = v
    scale = np.float32(H) ** -0.5
    qg = q.reshape(B, T, K, G, H)
    logits = np.einsum("btkgh,bskh->btskg", qg, kc) * scale
    logits = np.where(mask[:, :, :, None, None], logits, np.float32(-3.3895314e38))
    logits = logits - logits.max(axis=2, keepdims=True)
    w = np.exp(logits.astype(np.float32))
    w = w / w.sum(axis=2, keepdims=True)
    qkv = np.einsum("btskg,bskh->btkgh", w, vc).reshape(B, T, N, H)
    return np.einsum("btnh,nhd->btd", qkv, wo).astype(np.float32)


# ---------------------------------------------------------------- device build
def _build(sold):
    import concourse.bass as bass
    import concourse.bacc as bacc
    import concourse.tile as tile
    from concourse import mybir
    from concourse.masks import make_identity

    f32 = mybir.dt.float32
    bf = mybir.dt.bfloat16
    f8 = mybir.dt.float8e4
    NB = sold // 128      # 128-row s blocks per batch
    NQ = (NB + QD - 1) // QD
    BT = BL * T  # 32
    # one consolidated bf16 "smalls" tensor: swap matrix | ropeq | ropek | xT | nmask
    SW0, SW1 = 0, 128
    RQ0, RQ1 = 128, 128 + 2 * BT              # 192
    RK0, RK1 = RQ1, RQ1 + 2 * BT              # 256
    XT0, XT1 = RK1, RK1 + DC * BT             # 640
    NM0, NM1 = XT1, XT1 + BL * G * T          # 832
    SM = NM1

    nc = bacc.Bacc(None)
    smalld = nc.declare_dram_parameter("smalls", [128, SM], bf, isOutput=False)
    wqk = nc.declare_dram_parameter("wqk", [128, DC, G * H], bf, isOutput=False)
    wkv = nc.declare_dram_parameter("wkv", [128, DC, 2 * H], f8, isOutput=False)
    wok = nc.declare_dram_parameter("wok", [128, G, D], bf, isOutput=False)
    kcp = nc.declare_dram_parameter("kcp", [BL, 128, sold], f8, isOutput=False)
    vcp = nc.declare_dram_parameter("vcp", [BL, 128, NB, H + 1], bf, isOutput=False)
    outp = nc.declare_dram_parameter("out", [BT, D], f32, isOutput=True)

    with tile.TileContext(nc) as tc:
        with (
            tc.tile_pool(name="cpool", bufs=1) as cpool,
            tc.tile_pool(name="wtpool", bufs=3) as wtp,
            tc.tile_pool(name="spool", bufs=2) as sp,
            tc.tile_pool(name="pl", bufs=3, space="PSUM") as pl,
            tc.tile_pool(name="pacc", bufs=1, space="PSUM") as pacc,
            tc.tile_pool(name="pp", bufs=3, space="PSUM") as pp,
        ):
            ident = cpool.tile([128, 128], bf)
            make_identity(nc, ident)

            # ---- ALL loads on the SP HWDGE ring, strictly in need order,
            # consolidated to few triggers (each costs ~650ns on the SP seq).
            smalls = cpool.tile([128, SM], bf)
            nc.sync.dma_start(out=smalls, in_=smalld[:])
            swp = smalls[:, SW0:SW1]
            rq_s = smalls[:, RQ0:RQ1].rearrange("p (a t) -> p a t", a=2)
            rk_t = smalls[:, RK0:RK1].rearrange("p (a t) -> p a t", a=2)
            xT = smalls[:, XT0:XT1].rearrange("p (c t) -> p c t", c=DC)
            nm_t = smalls[0:T, NM0:NM1].rearrange("p (l gt) -> p l gt", l=BL)
            # wq chunked so projections start as soon as the first chunk lands
            wq_t = cpool.tile([128, DC, G * H], bf)
            for c0, c1 in ((0, 2), (2, 4), (4, 8), (8, 12)):
                nc.sync.dma_start(out=wq_t[:, c0:c1, :], in_=wqk[:, c0:c1, :])
            kT_all = cpool.tile([128, BL, sold], f8)
            vB_all = cpool.tile([128, BL, NB, H + 1], bf)
            wkv_t = cpool.tile([128, DC, 2 * H], f8)
            nc.sync.dma_start(out=wkv_t, in_=wkv[:])
            wk_t = wkv_t[:, :, 0:H]
            wv_t = wkv_t[:, :, H:2 * H]
            nc.sync.dma_start(out=kT_all[:, 0, :], in_=kcp[0])
            nc.sync.dma_start(out=vB_all[:, 0, :, :], in_=vcp[0])
            nc.sync.dma_start(out=kT_all[:, 1, :], in_=kcp[1])
            nc.sync.dma_start(out=vB_all[:, 1, :, :], in_=vcp[1])
            # wo halves: output projection consumes as chunks land
            wo_t = cpool.tile([128, G, D], bf)
            for cc in range(2):
                nc.sync.dma_start(out=wo_t[:, 3 * cc:3 * cc + 3, :],
                                  in_=wok[:, 3 * cc:3 * cc + 3, :])

            # ---- preload the Act EXP table off the critical path
            scr = cpool.tile([1, 4], f32)
            nc.vector.memset(scr, 0.0)
            scrE = cpool.tile([1, 4], bf)
            nc.scalar.activation(scrE, scr, mybir.ActivationFunctionType.Exp)

            # rope-q coeffs broadcast across heads (DVE is idle this early)
            rq_t = cpool.tile([128, 2, G, BT], bf)
            for g in range(G):
                nc.vector.tensor_copy(rq_t[:, :, g, :], rq_s)

            # ---- projections (accumulate over DC chunks)
            qp0 = pp.tile([BT, 512], f32, tag="pp")
            qp1 = pp.tile([BT, 256], f32, tag="pp")
            for c in range(DC):
                st, spf = (c == 0), (c == DC - 1)
                nc.tensor.matmul(qp0, xT[:, c, :], wq_t[:, c, 0:512], start=st, stop=spf)
                nc.tensor.matmul(qp1, xT[:, c, :], wq_t[:, c, 512:768], start=st, stop=spf)
            q_sb = cpool.tile([BT, G * H], bf)
            # ---- q -> qT, half-swap via PE permutation, RoPE (scale in coeffs)
            # per-head copies + transposes so the chain pipelines across
            # DVE and PE instead of serializing on full-tensor copies
            qTr = cpool.tile([128, G, BT], bf)
            for g in range(G):
                src = qp0[:, g * H:(g + 1) * H] if g < 4 else                     qp1[:, (g - 4) * H:(g - 3) * H]
                nc.vector.tensor_copy(q_sb[:, g * H:(g + 1) * H], src)
                tp = pl.tile([128, BT], bf, tag="pl")
                nc.tensor.transpose(tp, q_sb[:, g * H:(g + 1) * H], ident[:BT, :BT])
                nc.vector.tensor_copy(qTr[:, g, :], tp)
            qR = cpool.tile([128, G, BT], bf)
            qtmp = cpool.tile([128, G, BT], bf)
            nc.vector.tensor_tensor(qtmp, qTr, rq_t[:, 0], mybir.AluOpType.mult)
            qSwP = pl.tile([128, G * BT], f32, tag="pl", name="qswp")
            nc.tensor.matmul(qSwP, swp, qTr.rearrange("h g t -> h (g t)"),
                             start=True, stop=True)
            nc.vector.tensor_tensor(
                qR, qSwP.rearrange("h (g t) -> h g t", g=G), rq_t[:, 1],
                mybir.AluOpType.mult)
            nc.vector.tensor_tensor(qR, qR, qtmp, mybir.AluOpType.add)

            # ---- attention hot loop: per 128-row s block one logits matmul
            # (pre-transposed K stationary) and one qkv accumulate; exp runs
            # once per quad of blocks. PE issue is software-pipelined: quad
            # q's logits go out before quad q-1's qkv so PE never waits on Act.
            # The k/v projections + new-token block run between the two batch
            # loops (their wkv weights land after the caches in the stream);
            # the new-token matmuls close each accumulation group at the end.
            qkvT = cpool.tile([128, G, BT], bf)
            qkvPs = [pacc.tile([G * T, H + 1], f32, tag="pacc", name=f"qkvP{lb}")
                     for lb in range(BL)]

            def hot_loop(lb):
                qkvP = qkvPs[lb]
                qrs = qR[:, :, lb * T:(lb + 1) * T]
                pend = []   # [(wTq, q0, nqd)] awaiting qkv, depth 2
                first = [True]

                def flush_one():
                    pw, p0, pn = pend.pop(0)
                    for j in range(pn):
                        nc.tensor.matmul(qkvP, pw[:, j, :],
                                         vB_all[:, lb, p0 + j, :],
                                         start=first[0] and j == 0, stop=False,
                                         skip_group_check=True)
                    first[0] = False

                for q in range(NQ):
                    q0 = q * QD
                    nqd = min(QD, NB - q0)
                    lps = pl.tile([128, QD, G * T], f32, tag="pl")
                    for j in range(nqd):
                        blk = q0 + j
                        nc.tensor.matmul(lps[:, j, :],
                                         kT_all[:, lb, blk * 128:(blk + 1) * 128],
                                         qrs, start=True, stop=True,
                                         skip_group_check=True)
                    wTq = wtp.tile([128, QD, G * T], bf, tag="wt")
                    if nqd == QD:
                        nc.scalar.activation(wTq, lps,
                                             mybir.ActivationFunctionType.Exp)
                    else:
                        nc.scalar.activation(wTq[:, 0:nqd, :], lps[:, 0:nqd, :],
                                             mybir.ActivationFunctionType.Exp)
                    pend.append((wTq, q0, nqd))
                    if len(pend) > 2:
                        flush_one()
                while pend:
                    flush_one()

            # ---- k_new projection -> kT + RoPE (no scale)
            kp = pp.tile([BT, H], f32, tag="pp")
            for c in range(DC):
                nc.tensor.matmul(kp, xT[:, c, :], wk_t[:, c, :],
                                 start=(c == 0), stop=(c == DC - 1))
            k_sb = cpool.tile([BT, H], bf)
            nc.vector.tensor_copy(k_sb, kp)
            kTr = cpool.tile([128, BT], bf)
            tpk = pl.tile([128, BT], bf, tag="pl")
            nc.tensor.transpose(tpk, k_sb, ident[:BT, :BT])
            nc.vector.tensor_copy(kTr, tpk)
            kSwP = pl.tile([128, BT], f32, tag="pl", name="kswp")
            nc.tensor.matmul(kSwP, swp, kTr, start=True, stop=True)
            kR = cpool.tile([128, BT], bf)
            ktmp = cpool.tile([128, BT], bf)
            nc.vector.tensor_tensor(ktmp, kTr, rk_t[:, 0], mybir.AluOpType.mult)
            nc.vector.tensor_tensor(kR, kSwP, rk_t[:, 1], mybir.AluOpType.mult)
            nc.vector.tensor_tensor(kR, kR, ktmp, mybir.AluOpType.add)

            # ---- v_new projection
            vN = cpool.tile([T, BL, H + 1], bf)
            for lb in range(BL):
                vp = pp.tile([T, H], f32, tag="pp", name=f"vp{lb}")
                for c in range(DC):
                    nc.tensor.matmul(vp, xT[:, c, lb * T:(lb + 1) * T], wv_t[:, c, :],
                                     start=(c == 0), stop=(c == DC - 1))
                nc.vector.tensor_copy(vN[:, lb, 0:H], vp)
            nc.vector.memset(vN[:, :, H:H + 1], 1.0)

            # ---- new-token logits (kv rows [cur, cur+T) live on-chip)
            wTns = []
            for lb in range(BL):
                lpn = pl.tile([T, G * T], f32, tag="pl", name=f"lpn{lb}")
                nc.tensor.matmul(lpn, kR[:, lb * T:(lb + 1) * T],
                                 qR[:, :, lb * T:(lb + 1) * T],
                                 start=True, stop=True)
                nc.vector.tensor_tensor(lpn, lpn, nm_t[:, lb, :], mybir.AluOpType.add)
                wTn = sp.tile([T, G * T], bf, tag="wtn", name=f"wTn{lb}")
                nc.scalar.activation(wTn, lpn, mybir.ActivationFunctionType.Exp)
                wTns.append(wTn)

            hot_loop(0)

            hot_loop(1)

            # new tokens close both accumulation groups
            for lb in range(BL):
                nc.tensor.matmul(qkvPs[lb], wTns[lb], vN[:, lb, :],
                                 start=False, stop=True, skip_group_check=True)

            # ---- epilogues after both batches
            for lb in range(BL):
                qkvP = qkvPs[lb]
                rec = sp.tile([G * T, 1], f32, tag="rec")
                nc.vector.reciprocal(rec, qkvP[:, H:H + 1])
                qkvN = sp.tile([G * T, H], bf, tag="qkvN")
                nc.vector.tensor_scalar_mul(qkvN, qkvP[:, 0:H], rec)
                tp3 = pl.tile([128, G * T], bf, tag="pl")
                nc.tensor.transpose(tp3, qkvN, ident[:G * T, :G * T])
                nc.vector.tensor_copy(
                    qkvT[:, :, lb * T:(lb + 1) * T],
                    tp3.rearrange("h (g t) -> h g t", g=G))

            # ---- output projection: out[bt, d] = sum_g qkvT[h,g,bt]^T wo[h,g,d]
            o_sb = cpool.tile([BT, D], f32)
            for db in range(3):
                oP = pp.tile([BT, 512], f32, tag="pp", name=f"oP{db}")
                for g in range(G):
                    nc.tensor.matmul(oP, qkvT[:, g, :],
                                     wo_t[:, g, db * 512:(db + 1) * 512],
                                     start=(g == 0), stop=(g == G - 1))
                if db == 1:
                    nc.scalar.activation(o_sb[:, db * 512:(db + 1) * 512], oP,
                                         mybir.ActivationFunctionType.Copy)
                else:
                    nc.vector.tensor_copy(o_sb[:, db * 512:(db + 1) * 512], oP)
                nc.sync.dma_start(out=outp[:, db * 512:(db + 1) * 512],
                                  in_=o_sb[:, db * 512:(db + 1) * 512])

    nc.compile()  # bacc passes: splits multi-wait instructions (TRN2 allows 1)
    return nc


# ---------------------------------------------------------------- entry point
def kernel(x, k_cache, v_cache, wq, bq, wk, bk, wv, bv, wo,
           segment_ids, start_ind, cur_ind):
    x = np.asarray(x, np.float32)
    k_cache = np.asarray(k_cache, np.float32)
    v_cache = np.asarray(v_cache, np.float32)
    wq = np.asarray(wq, np.float32)
    wk = np.asarray(wk, np.float32)
    wv = np.asarray(wv, np.float32)
    wo = np.asarray(wo, np.float32)
    cur = int(np.asarray(cur_ind))

    mask, positions = _host_mask(segment_ids, start_ind, cur)

    spec_ok = (
        cur % 128 == 0 and 0 < cur and cur + T <= S
        and not np.any(np.asarray(bq)) and not np.any(np.asarray(bk))
        and not np.any(np.asarray(bv))
        and not np.any(mask[:, :, cur + T:])          # nothing attended past new rows
        and bool(np.all(np.any(mask, axis=2)))        # no fully-masked query row
        and bool(np.all(mask[:, :, :cur]))            # all old-cache rows attended
    )
    if not spec_ok:
        return _numpy_reference(x, k_cache, v_cache, wq, bq, wk, bk, wv, bv, wo,
                                segment_ids, start_ind, cur)

    sold = cur
    key = sold
    if key not in _built:
        _built[key] = _build(sold)
    nc = _built[key]

    inputs = dict(x=x, k_cache=k_cache, v_cache=v_cache, wq=wq, wk=wk, wv=wv,
                  wo=wo, segment_ids=segment_ids, start_ind=start_ind,
                  cur_ind=cur)
    in_maps = _make_in_maps(inputs, sold, mask=mask, positions=positions)

    global _last_in_maps
    _last_in_maps = in_maps

    import os
    from concourse.bass_utils import run_bass_kernel_spmd
    trace = os.environ.get("KERNEL_TRACE", "0") == "1"
    res = run_bass_kernel_spmd(nc, in_maps, core_ids=list(range(8)), trace=trace)
    if trace and res.exec_time_ns is not None:
        print(f"HW exec time: {res.exec_time_ns} ns")

    out = np.zeros((B, T, D), np.float32)
    for c in range(8):
        bg = c // 2
        out[bg * BL:(bg + 1) * BL] += res.results[c]["out"].reshape(BL, T, D)
    return out


def _bf(a):
    return np.ascontiguousarray(a, dtype=BF16)


def _make_in_maps(inputs, sold, mask=None, positions=None):
    x = np.asarray(inputs["x"], np.float32)
    k_cache = np.asarray(inputs["k_cache"], np.float32)
    v_cache = np.asarray(inputs["v_cache"], np.float32)
    wq = np.asarray(inputs["wq"], np.float32)
    wk = np.asarray(inputs["wk"], np.float32)
    wv = np.asarray(inputs["wv"], np.float32)
    wo = np.asarray(inputs["wo"], np.float32)
    cur = int(np.asarray(inputs["cur_ind"]))
    NB = sold // 128
    BT = BL * T
    if mask is None:
        mask, positions = _host_mask(inputs["segment_ids"], inputs["start_ind"], cur)

    sin, cos = _host_rope(positions)  # [b, t, 64]
    scale = np.float32(H ** -0.5)

    # rope coeff layouts: rows h<64 -> (cos, -sin); h>=64 -> (cos, +sin)
    def rope_pack(bsl, ncols_g, with_scale):
        # returns [128, 2, ncols_g, BL*T]
        cs = cos[bsl]  # [BL, T, 64]
        sn = sin[bsl]
        ccol = np.transpose(cs, (2, 0, 1)).reshape(64, BL * T)  # [64, (b,t)]
        scol = np.transpose(sn, (2, 0, 1)).reshape(64, BL * T)
        top_c, bot_c = ccol, ccol
        top_s, bot_s = -scol, scol
        c128 = np.concatenate([top_c, bot_c], axis=0)   # [128, BT]
        s128 = np.concatenate([top_s, bot_s], axis=0)
        if with_scale:
            c128 = c128 * scale
            s128 = s128 * scale
        pack = np.stack([c128, s128], axis=1)           # [128, 2, BT]
        pack = np.repeat(pack[:, :, None, :], ncols_g, axis=2)
        return _bf(pack)

    # half-swap permutation: out[p, :] = in[(p + 64) % 128, :]
    swpm = np.zeros((128, 128), np.float32)
    swpm[(np.arange(128) + 64) % 128, np.arange(128)] = 1.0

    ones_col = np.ones((BL, sold, 1), np.float32)
    in_maps = []
    for c in range(8):
        k = c % 2
        bg = c // 2
        bsl = slice(bg * BL, (bg + 1) * BL)
        # x pre-transposed to contraction-major: [128, DC, BT]
        xT = x[bsl].reshape(BT, DC, 128).transpose(2, 1, 0)
        # weights in SBUF layout (partition = contraction chunk row)
        wq4 = wq.reshape(DC, 128, N, H)[:, :, k * G:(k + 1) * G, :] \
                .reshape(DC, 128, G * H).transpose(1, 0, 2)
        wk4 = wk.reshape(DC, 128, K, H)[:, :, k, :].transpose(1, 0, 2)
        wv4 = wv.reshape(DC, 128, K, H)[:, :, k, :].transpose(1, 0, 2)
        wkv4 = np.concatenate([wk4, wv4], axis=-1)       # [128, DC, 2H]
        wo4 = wo[k * G:(k + 1) * G].transpose(1, 0, 2)   # [128h, G, D]
        # K cache pre-transposed to [BL, H, sold]
        kcs = k_cache[bsl, :sold, k, :].transpose(0, 2, 1)
        # V cache block-major: [BL, 128, NB, H+1], s = blk*128 + p
        vcs = np.concatenate([v_cache[bsl, :sold, k, :], ones_col], axis=-1)
        vcs = vcs.reshape(BL, NB, 128, H + 1).transpose(0, 2, 1, 3)
        # additive mask for the new-token block: [T(s_new), BL, G*T]
        nm = np.where(mask[bsl][:, :, cur:cur + T], np.float32(0), np.float32(NEG))
        nm = np.transpose(nm, (2, 0, 1))                 # [s_new, BL, t]
        nm = np.repeat(nm[:, :, None, :], G, axis=2).reshape(T, BL, G * T)
        # consolidated smalls pack [128, SM] bf16
        BT_ = BL * T
        smalls = np.zeros((128, 832), np.float32)
        smalls[:, 0:128] = swpm
        smalls[:, 128:192] = np.asarray(
            rope_pack(bsl, 1, True), np.float32).reshape(128, 2 * BT_)
        smalls[:, 192:256] = np.asarray(
            rope_pack(bsl, 1, False), np.float32).reshape(128, 2 * BT_)
        smalls[:, 256:640] = xT.reshape(128, DC * BT_)
        smalls[0:T, 640:832] = nm.reshape(T, BL * G * T)
        in_maps.append({
            "smalls": _bf(smalls),
            "wqk": _bf(wq4),
            "wkv": np.ascontiguousarray(wkv4, dtype=FP8),
            "wok": _bf(wo4),
            "kcp": np.ascontiguousarray(kcs, dtype=FP8),
            "vcp": _bf(vcs),
        })

    return in_maps


# revision 28
# speedup vs baseline: 1.1244x; 1.0181x over previous
"""GQA attention decode kernel for Trainium2 (Bass/Tile), SPMD over 8 NeuronCores.

Sharding: kv-head axis (K=2) x batch groups (4) -> 8 cores.
Core c: kv head k=c%2, batches [2*(c//2), 2*(c//2)+2).
Each core computes q/k/v projections + RoPE for its head group, attends over
its shard of the KV cache (only rows [0, cur_ind+T) ever contribute), and
produces a partial output projection. Host sums the two kv-head partials.

Heavy tensors stream as bf16, and the K cache + wk/wv as fp8 e4m3 (the
only fp8 placements whose quantization noise stays well under the rel-err
budget; wq/wo/V-cache must stay bf16). PSUM accumulation stays fp32. The
K cache is pre-transposed on host to [H, S] per batch so the hot loop
needs no PE transposes; the V cache is packed block-major [128, NB, H+1]
with a ones column folded in (softmax denominator accumulates alongside
the numerator in the same matmul chain).

Every DMA rides the single SP HWDGE ring in strict need order (ring order
== completion order), so compute overlaps the stream: smalls (swap matrix
/ rope coeffs / xT / new-token mask) first, then wq in chunks (projections
start as chunk 0 lands), wkv, batch-0 cache, batch-1 cache, wo halves
(output projection consumes them at the end). Exp is batched 4 s-blocks
per activation to amortize the ~175ns fixed Act-engine cost, with PE issue
software-pipelined two quads deep so PE never waits on Act. The RoPE
half-swap is a PE permutation matmul (no SBUF->SBUF DMA on the hot ring).
The k/v projections + new-token logits sit between the q-projection and
the batch-0 hot loop, hiding under the cache stream; the new-token qkv
matmuls close each batch's PSUM accumulation group after both hot loops.

Shapes (hardcoded from the problem spec):
  x [8,16,1536], k_cache/v_cache [8,8192,2,128],
  wq [1536,12,128], wk/wv [1536,2,128], wo [12,128,1536], out [8,16,1536]
"""

import sys

if "/opt/trn_rl_repo" not in sys.path:
    sys.path.insert(0, "/opt/trn_rl_repo")

import numpy as np
import ml_dtypes

BF16 = np.dtype(ml_dtypes.bfloat16)
FP8 = np.dtype(ml_dtypes.float8_e4m3)

B, T, S, D, N, K, H = 8, 16, 8192, 1536, 12, 2, 128
G = N // K            # 6 q heads per kv head
BG = 4                # batch groups
BL = B // BG          # 2 local batches per core
DC = D // 128         # 12 contraction chunks
QD = 4                # s-blocks per exp batch (4*96 fp32 = 1.5KB of a PSUM bank)
ROPE_THETA = 1000000.0
NEG = -1.0e30

_built = {}


# ---------------------------------------------------------------- host math
def _host_rope(positions):
    # positions [b, t] int32 -> sin, cos [b, t, 64] float32 (mirrors reference)
    frac = np.arange(0, H, 2, dtype=np.float32) / np.float32(H)
    timescale = np.power(np.float32(ROPE_THETA), frac, dtype=np.float32)
    ang = positions[..., None].astype(np.float32) / timescale
    return np.sin(ang, dtype=np.float32), np.cos(ang, dtype=np.float32)


def _host_mask(segment_ids, start_ind, cur):
    seg = np.asarray(segment_ids, np.int32)
    sti = np.asarray(start_ind, np.int32)
    nonpad = seg != 0
    left_pads = np.argmax(nonpad, axis=-1).astype(np.int32)
    start = np.where(sti < 0, left_pads, sti).astype(np.int32)
    positions = np.maximum(np.cumsum(nonpad.astype(np.int32), axis=-1) - 1, 0) + cur

    q_pos = cur + np.arange(T, dtype=np.int32)[None, :] - start[:, None]
    ts_ = np.arange(S, dtype=np.int32)
    kv_seg = (ts_[None, :] >= start[:, None]) & (ts_[None, :] < cur + T)
    k_pos = ts_[None, :] - start[:, None]
    causal = k_pos[:, None, :] <= q_pos[:, :, None]
    segm = kv_seg[:, None, :].astype(np.int32) == seg[:, :, None]
    mask = causal & segm  # [b, t, S] True = attend
    return mask, positions


def _numpy_reference(x, k_cache, v_cache, wq, bq, wk, bk, wv, bv, wo,
                     segment_ids, start_ind, cur):
    # Full-precision numpy fallback (used only for inputs outside the
    # spec envelope: non-zero biases, odd cur_ind alignment, pad tokens).
    mask, positions = _host_mask(segment_ids, start_ind, cur)
    sin, cos = _host_rope(positions)

    def rope(t):  # t [b,tk,n,h]
        h2 = H // 2
        x1, x2 = t[..., :h2], t[..., h2:]
        s = sin[:, :, None, :]
        c = cos[:, :, None, :]
        return np.concatenate([x1 * c - x2 * s, x2 * c + x1 * s], axis=-1)

    q = np.einsum("btd,dnh->btnh", x, wq) + bq
    kp = np.einsum("btd,dkh->btkh", x, wk) + bk
    v = np.einsum("btd,dkh->btkh", x, wv) + bv
    q = rope(q)
    kp = rope(kp)
    kc = np.array(k_cache)
    vc = np.array(v_cache)
    kc[:, cur:cur + T] = kp
    vc[:, cur:cur + T] = v
    scale = np.float32(H) ** -0.5
    qg = q.reshape(B, T, K, G, H)
    logits = np.einsum("btkgh,bskh->btskg", qg, kc) * scale
    logits = np.where(mask[:, :, :, None, None], logits, np.float32(-3.3895314e38))
    logits = logits - logits.max(axis=2, keepdims=True)
    w = np.exp(logits.astype(np.float32))
    w = w / w.sum(axis=2, keepdims=True)
    qkv = np.einsum("btskg,bskh->btkgh", w, vc).reshape(B, T, N, H)
    return np.einsum("btnh,nhd->btd", qkv, wo).astype(np.float32)


# ---------------------------------------------------------------- device build
def _build(sold):
    import concourse.bass as bass
    import concourse.bacc as bacc
    import concourse.tile as tile
    from concourse import mybir
    from concourse.masks import make_identity

    f32 = mybir.dt.float32
    bf = mybir.dt.bfloat16
    f8 = mybir.dt.float8e4
    NB = sold // 128      # 128-row s blocks per batch
    NQ = (NB + QD - 1) // QD
    BT = BL * T  # 32
    # one consolidated bf16 "smalls" tensor: swap matrix | ropeq | ropek | xT | nmask
    SW0, SW1 = 0, 128
    RQ0, RQ1 = 128, 128 + 2 * BT              # 192
    RK0, RK1 = RQ1, RQ1 + 2 * BT              # 256
    XT0, XT1 = RK1, RK1 + DC * BT             # 640
    NM0, NM1 = XT1, XT1 + BL * G * T          # 832
    SM = NM1

    nc = bacc.Bacc(None)
    smalld = nc.declare_dram_parameter("smalls", [128, SM], bf, isOutput=False)
    wqk = nc.declare_dram_parameter("wqk", [128, DC, G * H], bf, isOutput=False)
    wkv = nc.declare_dram_parameter("wkv", [128, DC, 2 * H], f8, isOutput=False)
    wok = nc.declare_dram_parameter("wok", [128, G, D], bf, isOutput=False)
    kcp = nc.declare_dram_parameter("kcp", [BL, 128, sold], f8, isOutput=False)
    vcp = nc.declare_dram_parameter("vcp", [BL, 128, NB, H + 1], bf, isOutput=False)
    outp = nc.declare_dram_parameter("out", [BT, D], f32, isOutput=True)

    with tile.TileContext(nc) as tc:
        with (
            tc.tile_pool(name="cpool", bufs=1) as cpool,
            tc.tile_pool(name="wtpool", bufs=3) as wtp,
            tc.tile_pool(name="spool", bufs=2) as sp,
            tc.tile_pool(name="pl", bufs=3, space="PSUM") as pl,
            tc.tile_pool(name="pacc", bufs=1, space="PSUM") as pacc,
            tc.tile_pool(name="pp", bufs=3, space="PSUM") as pp,
        ):
            ident = cpool.tile([128, 128], bf)
            make_identity(nc, ident)

            # ---- ALL loads on the SP HWDGE ring, strictly in need order,
            # consolidated to few triggers (each costs ~650ns on the SP seq).
            smalls = cpool.tile([128, SM], bf)
            nc.sync.dma_start(out=smalls, in_=smalld[:])
            swp = smalls[:, SW0:SW1]
            rq_s = smalls[:, RQ0:RQ1].rearrange("p (a t) -> p a t", a=2)
            rk_t = smalls[:, RK0:RK1].rearrange("p (a t) -> p a t", a=2)
            xT = smalls[:, XT0:XT1].rearrange("p (c t) -> p c t", c=DC)
            nm_t = smalls[0:T, NM0:NM1].rearrange("p (l gt) -> p l gt", l=BL)
            # wq chunked so projections start as soon as the first chunk lands
            wq_t = cpool.tile([128, DC, G * H], bf)
            for c0, c1 in ((0, 2), (2, 4), (4, 8), (8, 12)):
                nc.sync.dma_start(out=wq_t[:, c0:c1, :], in_=wqk[:, c0:c1, :])
            kT_all = cpool.tile([128, BL, sold], f8)
            vB_all = cpool.tile([128, BL, NB, H + 1], bf)
            wkv_t = cpool.tile([128, DC, 2 * H], f8)
            nc.sync.dma_start(out=wkv_t, in_=wkv[:])
            wk_t = wkv_t[:, :, 0:H]
            wv_t = wkv_t[:, :, H:2 * H]
            nc.sync.dma_start(out=kT_all[:, 0, :], in_=kcp[0])
            nc.sync.dma_start(out=vB_all[:, 0, :, :], in_=vcp[0])
            nc.sync.dma_start(out=kT_all[:, 1, :], in_=kcp[1])
            nc.sync.dma_start(out=vB_all[:, 1, :, :], in_=vcp[1])
            # wo halves: output projection consumes as chunks land
            wo_t = cpool.tile([128, G, D], bf)
            for cc in range(2):
                nc.sync.dma_start(out=wo_t[:, 3 * cc:3 * cc + 3, :],
                                  in_=wok[:, 3 * cc:3 * cc + 3, :])

            # ---- preload the Act EXP table off the critical path
            scr = cpool.tile([1, 4], f32)
            nc.vector.memset(scr, 0.0)
            scrE = cpool.tile([1, 4], bf)
            nc.scalar.activation(scrE, scr, mybir.ActivationFunctionType.Exp)

            # rope-q coeffs broadcast across heads (DVE is idle this early)
            rq_t = cpool.tile([128, 2, G, BT], bf)
            for g in range(G):
                nc.vector.tensor_copy(rq_t[:, :, g, :], rq_s)

            # ---- projections (accumulate over DC chunks)
            qp0 = pp.tile([BT, 512], f32, tag="pp")
            qp1 = pp.tile([BT, 256], f32, tag="pp")
            for c in range(DC):
                st, spf = (c == 0), (c == DC - 1)
                nc.tensor.matmul(qp0, xT[:, c, :], wq_t[:, c, 0:512], start=st, stop=spf)
                nc.tensor.matmul(qp1, xT[:, c, :], wq_t[:, c, 512:768], start=st, stop=spf)
            q_sb = cpool.tile([BT, G * H], bf)
            # ---- q -> qT, half-swap via PE permutation, RoPE (scale in coeffs)
            # per-head copies + transposes so the chain pipelines across
            # DVE and PE instead of serializing on full-tensor copies
            qTr = cpool.tile([128, G, BT], bf)
            for g in range(G):
                src = qp0[:, g * H:(g + 1) * H] if g < 4 else                     qp1[:, (g - 4) * H:(g - 3) * H]
                nc.vector.tensor_copy(q_sb[:, g * H:(g + 1) * H], src)
                tp = pl.tile([128, BT], bf, tag="pl")
                nc.tensor.transpose(tp, q_sb[:, g * H:(g + 1) * H], ident[:BT, :BT])
                nc.vector.tensor_copy(qTr[:, g, :], tp)
            qR = cpool.tile([128, G, BT], bf)
            qtmp = cpool.tile([128, G, BT], bf)
            nc.vector.tensor_tensor(qtmp, qTr, rq_t[:, 0], mybir.AluOpType.mult)
            qSwP = pl.tile([128, G * BT], f32, tag="pl", name="qswp")
            nc.tensor.matmul(qSwP, swp, qTr.rearrange("h g t -> h (g t)"),
                             start=True, stop=True)
            nc.vector.tensor_tensor(
                qR, qSwP.rearrange("h (g t) -> h g t", g=G), rq_t[:, 1],
                mybir.AluOpType.mult)
            nc.vector.tensor_tensor(qR, qR, qtmp, mybir.AluOpType.add)

            # ---- attention hot loop: per 128-row s block one logits matmul
            # (pre-transposed K stationary) and one qkv accumulate; exp runs
            # once per quad of blocks. PE issue is software-pipelined: quad
            # q's logits go out before quad q-1's qkv so PE never waits on Act.
            # The k/v projections + new-token block run between the two batch
            # loops (their wkv weights land after the caches in the stream);
            # the new-token matmuls close each accumulation group at the end.
            qkvT = cpool.tile([128, G, BT], bf)
            qkvPs = [pacc.tile([G * T, H + 1], f32, tag="pacc", name=f"qkvP{lb}")
                     for lb in range(BL)]

            def hot_loop(lb):
                qkvP = qkvPs[lb]
                qrs = qR[:, :, lb * T:(lb + 1) * T]
                pend = []   # [(wTq, q0, nqd)] awaiting qkv, depth 2
                first = [True]

                def flush_one():
                    pw, p0, pn = pend.pop(0)
                    for j in range(pn):
                        nc.tensor.matmul(qkvP, pw[:, j, :],
                                         vB_all[:, lb, p0 + j, :],
                                         start=first[0] and j == 0, stop=False,
                                         skip_group_check=True)
                    first[0] = False

                for q in range(NQ):
                    q0 = q * QD
                    nqd = min(QD, NB - q0)
                    lps = pl.tile([128, QD, G * T], f32, tag="pl")
                    for j in range(nqd):
                        blk = q0 + j
                        nc.tensor.matmul(lps[:, j, :],
                                         kT_all[:, lb, blk * 128:(blk + 1) * 128],
                                         qrs, start=True, stop=True,
                                         skip_group_check=True)
                    wTq = wtp.tile([128, QD, G * T], bf, tag="wt")
                    if nqd == QD:
                        nc.scalar.activation(wTq, lps,
                                             mybir.ActivationFunctionType.Exp)
                    else:
                        nc.scalar.activation(wTq[:, 0:nqd, :], lps[:, 0:nqd, :],
                                             mybir.ActivationFunctionType.Exp)
                    pend.append((wTq, q0, nqd))
                    if len(pend) > 2:
                        flush_one()
                while pend:
                    flush_one()

            # ---- k_new projection -> kT + RoPE (no scale)
            kp = pp.tile([BT, H], f32, tag="pp")
            for c in range(DC):
                nc.tensor.matmul(kp, xT[:, c, :], wk_t[:, c, :],
                                 start=(c == 0), stop=(c == DC - 1))
            k_sb = cpool.tile([BT, H], bf)
            nc.vector.tensor_copy(k_sb, kp)
            kTr = cpool.tile([128, BT], bf)
            tpk = pl.tile([128, BT], bf, tag="pl")
            nc.tensor.transpose(tpk, k_sb, ident[:BT, :BT])
            nc.vector.tensor_copy(kTr, tpk)
            kSwP = pl.tile([128, BT], f32, tag="pl", name="kswp")
            nc.tensor.matmul(kSwP, swp, kTr, start=True, stop=True)
            kR = cpool.tile([128, BT], bf)
            ktmp = cpool.tile([128, BT], bf)
            nc.vector.tensor_tensor(ktmp, kTr, rk_t[:, 0], mybir.AluOpType.mult)
            nc.vector.tensor_tensor(kR, kSwP, rk_t[:, 1], mybir.AluOpType.mult)
            nc.vector.tensor_tensor(kR, kR, ktmp, mybir.AluOpType.add)

            # ---- v_new projection
            vN = cpool.tile([T, BL, H + 1], bf)
            for lb in range(BL):
                vp = pp.tile([T, H], f32, tag="pp", name=f"vp{lb}")
                for c in range(DC):
                    nc.tensor.matmul(vp, xT[:, c, lb * T:(lb + 1) * T], wv_t[:, c, :],
                                     start=(c == 0), stop=(c == DC - 1))
                nc.vector.tensor_copy(vN[:, lb, 0:H], vp)
            nc.vector.memset(vN[:, :, H:H + 1], 1.0)

            # ---- new-token logits (kv rows [cur, cur+T) live on-chip)
            wTns = []
            for lb in range(BL):
                lpn = pl.tile([T, G * T], f32, tag="pl", name=f"lpn{lb}")
                nc.tensor.matmul(lpn, kR[:, lb * T:(lb + 1) * T],
                                 qR[:, :, lb * T:(lb + 1) * T],
                                 start=True, stop=True)
                nc.vector.tensor_tensor(lpn, lpn, nm_t[:, lb, :], mybir.AluOpType.add)
                wTn = sp.tile([T, G * T], bf, tag="wtn", name=f"wTn{lb}")
                nc.scalar.activation(wTn, lpn, mybir.ActivationFunctionType.Exp)
                wTns.append(wTn)

            hot_loop(0)

            hot_loop(1)

            # new tokens close both accumulation groups
            for lb in range(BL):
                nc.tensor.matmul(qkvPs[lb], wTns[lb], vN[:, lb, :],
                                 start=False, stop=True, skip_group_check=True)

            # ---- epilogues after both batches
            for lb in range(BL):
                qkvP = qkvPs[lb]
                rec = sp.tile([G * T, 1], f32, tag="rec")
                nc.vector.reciprocal(rec, qkvP[:, H:H + 1])
                qkvN = sp.tile([G * T, H], bf, tag="qkvN")
                nc.vector.tensor_scalar_mul(qkvN, qkvP[:, 0:H], rec)
                tp3 = pl.tile([128, G * T], bf, tag="pl")
                nc.tensor.transpose(tp3, qkvN, ident[:G * T, :G * T])
                nc.vector.tensor_copy(
                    qkvT[:, :, lb * T:(lb + 1) * T],
                    tp3.rearrange("h (g t) -> h g t", g=G))

            # ---- output projection: out[bt, d] = sum_g qkvT[h,g,bt]^T wo[h,g,d]
            o_sb = cpool.tile([BT, D], f32)
            for db in range(3):
                oP = pp.tile([BT, 512], f32, tag="pp", name=f"oP{db}")
                for g in range(G):
                    nc.tensor.matmul(oP, qkvT[:, g, :],
                                     wo_t[:, g, db * 512:(db + 1) * 512],
                                     start=(g == 0), stop=(g == G - 1))
                if db == 1:
                    nc.scalar.activation(o_sb[:, db * 512:(db + 1) * 512], oP,
                                         mybir.ActivationFunctionType.Copy)
                else:
                    nc.vector.tensor_copy(o_sb[:, db * 512:(db + 1) * 512], oP)
                nc.sync.dma_start(out=outp[:, db * 512:(db + 1) * 512],
                                  in_=o_sb[:, db * 512:(db + 1) * 512])

    nc.compile()  # bacc passes: splits multi-wait instructions (TRN2 allows 1)
    return nc


# ---------------------------------------------------------------- entry point
def kernel(x, k_cache, v_cache, wq, bq, wk, bk, wv, bv, wo,
           segment_ids, start_ind, cur_ind):
    x = np.asarray(x, np.float32)
    k_cache = np.asarray(k_cache, np.float32)
    v_cache = np.asarray(v_cache, np.float32)
    wq = np.asarray(wq, np.float32)
    wk = np.asarray(wk, np.float32)
    wv = np.asarray(wv, np.float32)
    wo = np.asarray(wo, np.float32)
    cur = int(np.asarray(cur_ind))

    mask, positions = _host_mask(segment_ids, start_ind, cur)

    spec_ok = (
        cur % 128 == 0 and 0 < cur and cur + T <= S
        and not np.any(np.asarray(bq)) and not np.any(np.asarray(bk))
        and not np.any(np.asarray(bv))
        and not np.any(mask[:, :, cur + T:])          # nothing attended past new rows
        and bool(np.all(np.any(mask, axis=2)))        # no fully-masked query row
        and bool(np.all(mask[:, :, :cur]))            # all old-cache rows attended
    )
    if not spec_ok:
        return _numpy_reference(x, k_cache, v_cache, wq, bq, wk, bk, wv, bv, wo,
                                segment_ids, start_ind, cur)

    sold = cur
    key = sold
    if key not in _built:
        _built[key] = _build(sold)
    nc = _built[key]

    inputs = dict(x=x, k_cache=k_cache, v_cache=v_cache, wq=wq, wk=wk, wv=wv,
                  wo=wo, segment_ids=segment_ids, start_ind=start_ind,
                  cur_ind=cur)
    in_maps = _make_in_maps(inputs, sold, mask=mask, positions=positions)

    global _last_in_maps
    _last_in_maps = in_maps

    import os
    from concourse.bass_utils import run_bass_kernel_spmd
    trace = os.environ.get("KERNEL_TRACE", "0") == "1"
    res = run_bass_kernel_spmd(nc, in_maps, core_ids=list(range(8)), trace=trace)
    if trace and res.exec_time_ns is not None:
        print(f"HW exec time: {res.exec_time_ns} ns")

    out = np.zeros((B, T, D), np.float32)
    for c in range(8):
        bg = c // 2
        out[bg * BL:(bg + 1) * BL] += res.results[c]["out"].reshape(BL, T, D)
    return out


def _bf(a):
    return np.ascontiguousarray(a, dtype=BF16)


def _make_in_maps(inputs, sold, mask=None, positions=None):
    x = np.asarray(inputs["x"], np.float32)
    k_cache = np.asarray(inputs["k_cache"], np.float32)
    v_cache = np.asarray(inputs["v_cache"], np.float32)
    wq = np.asarray(inputs["wq"], np.float32)
    wk = np.asarray(inputs["wk"], np.float32)
    wv = np.asarray(inputs["wv"], np.float32)
    wo = np.asarray(inputs["wo"], np.float32)
    cur = int(np.asarray(inputs["cur_ind"]))
    NB = sold // 128
    BT = BL * T
    if mask is None:
        mask, positions = _host_mask(inputs["segment_ids"], inputs["start_ind"], cur)

    sin, cos = _host_rope(positions)  # [b, t, 64]
    scale = np.float32(H ** -0.5)

    # rope coeff layouts: rows h<64 -> (cos, -sin); h>=64 -> (cos, +sin)
    def rope_pack(bsl, ncols_g, with_scale):
        # returns [128, 2, ncols_g, BL*T]
        cs = cos[bsl]  # [BL, T, 64]
        sn = sin[bsl]
        ccol = np.transpose(cs, (2, 0, 1)).reshape(64, BL * T)  # [64, (b,t)]
        scol = np.transpose(sn, (2, 0, 1)).reshape(64, BL * T)
        top_c, bot_c = ccol, ccol
        top_s, bot_s = -scol, scol
        c128 = np.concatenate([top_c, bot_c], axis=0)   # [128, BT]
        s128 = np.concatenate([top_s, bot_s], axis=0)
        if with_scale:
            c128 = c128 * scale
            s128 = s128 * scale
        pack = np.stack([c128, s128], axis=1)           # [128, 2, BT]
        pack = np.repeat(pack[:, :, None, :], ncols_g, axis=2)
        return _bf(pack)

    # half-swap permutation: out[p, :] = in[(p + 64) % 128, :]
    swpm = np.zeros((128, 128), np.float32)
    swpm[(np.arange(128) + 64) % 128, np.arange(128)] = 1.0

    ones_col = np.ones((BL, sold, 1), np.float32)
    in_maps = []
    for c in range(8):
        k = c % 2
        bg = c // 2
        bsl = slice(bg * BL, (bg + 1) * BL)
        # x pre-transposed to contraction-major: [128, DC, BT]
        xT = x[bsl].reshape(BT, DC, 128).transpose(2, 1, 0)
        # weights in SBUF layout (partition = contraction chunk row)
        wq4 = wq.reshape(DC, 128, N, H)[:, :, k * G:(k + 1) * G, :] \
                .reshape(DC, 128, G * H).transpose(1, 0, 2)
        wk4 = wk.reshape(DC, 128, K, H)[:, :, k, :].transpose(1, 0, 2)
        wv4 = wv.reshape(DC, 128, K, H)[:, :, k, :].transpose(1, 0, 2)
        wkv4 = np.concatenate([wk4, wv4], axis=-1)       # [128, DC, 2H]
        wo4 = wo[k * G:(k + 1) * G].transpose(1, 0, 2)   # [128h, G, D]
        # K cache pre-transposed to [BL, H, sold]
        kcs = k_cache[bsl, :sold, k, :].transpose(0, 2, 1)
        # V cache block-major: [BL, 128, NB, H+1], s = blk*128 + p
        vcs = np.concatenate([v_cache[bsl, :sold, k, :], ones_col], axis=-1)
        vcs = vcs.reshape(BL, NB, 128, H + 1).transpose(0, 2, 1, 3)
        # additive mask for the new-token block: [T(s_new), BL, G*T]
        nm = np.where(mask[bsl][:, :, cur:cur + T], np.float32(0), np.float32(NEG))
        nm = np.transpose(nm, (2, 0, 1))                 # [s_new, BL, t]
        nm = np.repeat(nm[:, :, None, :], G, axis=2).reshape(T, BL, G * T)
        # consolidated smalls pack [128, SM] bf16
        BT_ = BL * T
        smalls = np.zeros((128, 832), np.float32)
        smalls[:, 0:128] = swpm
        smalls[:, 128:192] = np.asarray(
            rope_pack(bsl, 1, True), np.float32).reshape(128, 2 * BT_)
        smalls[:, 192:256] = np.asarray(
            rope_pack(bsl, 1, False), np.float32).reshape(128, 2 * BT_)
        smalls[:, 256:640] = xT.reshape(128, DC * BT_)
        smalls[0:T, 640:832] = nm.reshape(T, BL * G * T)
        in_maps.append({
            "smalls": _bf(smalls),
            "wqk": _bf(wq4),
            "wkv": np.ascontiguousarray(wkv4, dtype=FP8),
            "wok": _bf(wo4),
            "kcp": np.ascontiguousarray(kcs, dtype=FP8),
            "vcp": _bf(vcs),
        })

    return in_maps
